# revision 15
# baseline (speedup 1.0000x reference)
"""Trainium2 Bass kernel for nn_CustomLoss (cross-entropy + worst-class masked loss).

Computes: loss = mean_i(logsumexp(output_i) - output_i[target_i])
          result = loss * (1 + mean_i(target_i in {3,5,8,9}))

Data-parallel over 8 NeuronCores: each core streams its 32768x1000 f32 shard
as 4 MB chunks. Rows are pre-sorted by target on the host (the loss is
permutation-invariant), so each [128,1000] tile's targets fall inside a
narrow static column window; the target-logit gather is a cheap windowed DVE
scalar_tensor_tensor over 32 columns. Row-sums of exp are split within each
chunk between the ACT accumulator (NA tiles) and one multi-row ACT exp +
DVE 3D tensor_reduce (G-NA tiles), keeping both compute engines ~15% under
the DMA streaming pace so the DMA rings stay the pacer. The first and last
chunks are split into quarter-DMAs to shorten pipeline ramp and drain.
Host combines the per-core partial sums.
"""
import os
import numpy as np
from contextlib import ExitStack

import concourse.bacc as bacc
import concourse.tile as tile
from concourse import mybir
from concourse.bass_utils import run_bass_kernel_spmd

F32 = mybir.dt.float32
AF = mybir.ActivationFunctionType
ALU = mybir.AluOpType

N_CORES = 8
B, C = 262144, 1000
ROWS = B // N_CORES           # 32768 rows per core
P = 128                       # SBUF partitions
G = 8                         # [128, C] sub-tiles per DMA chunk
N_CHUNKS = ROWS // (P * G)    # 32 chunks of [128, G, C] (4 MB contiguous)
N_TILES = ROWS // P           # 256 logical [128, C] tiles
W = 32                        # gather window width (covers sorted-target spread)
NA = int(os.environ.get("K_NA", "3"))   # ACT-accum tiles per chunk (of G)
BUFS_X = int(os.environ.get("K_BUFS", "3"))
WORST = (3.0, 5.0, 8.0, 9.0)

_CACHE = {}


def _window_starts():
    """Static per-tile gather window start columns.

    After sorting, tile k (sorted rows 128k..128k+127) has targets centered
    near 1000*(128k+64)/32768; the spread across 8 cores is a few classes.
    A +/-16 window covers it with margin; _shard_inputs asserts this
    against the actual data.
    """
    los = []
    for k in range(N_TILES):
        center = (128 * k + 64) * C // ROWS
        lo = min(max(center - W // 2, 0), C - W)
        los.append(lo)
    return los

LOS = _window_starts()


def _build():
    nc = bacc.Bacc(None, target_bir_lowering=False, debug=False,
                   num_devices=N_CORES)
    x_h = nc.declare_dram_parameter("x", [N_CHUNKS, P, G, C], F32, isOutput=False)
    tgt_h = nc.declare_dram_parameter("tgt", [P, N_TILES], F32, isOutput=False)
    iota_h = nc.declare_dram_parameter("iota", [P, C], F32, isOutput=False)
    out_h = nc.declare_dram_parameter("out", [P, 2], F32, isOutput=True)

    with tile.TileContext(nc) as tc, ExitStack() as ctx:
        xp = ctx.enter_context(tc.tile_pool(name="xp", bufs=BUFS_X))
        xh = ctx.enter_context(tc.tile_pool(name="xh", bufs=4))
        scr = ctx.enter_context(tc.tile_pool(name="scr", bufs=2))
        em = ctx.enter_context(tc.tile_pool(name="em", bufs=2))
        pers = ctx.enter_context(tc.tile_pool(name="pers", bufs=1))

        s_cols = pers.tile([P, N_TILES], F32, tag="s_cols")   # sum_j exp(x_ij)
        g_cols = pers.tile([P, N_TILES], F32, tag="g_cols")   # x_i[t_i]
        tgt_sb = pers.tile([P, N_TILES], F32, tag="tgt_sb")
        iota_sb = pers.tile([P, C], F32, tag="iota_sb")
        fin = pers.tile([P, 8], F32, tag="fin")
        out_sb = pers.tile([P, 2], F32, tag="out_sb")

        def gather(x_t, j, k):
            lo = LOS[k]
            m_scr = scr.tile([P, W], F32, tag="m_scr")
            nc.vector.scalar_tensor_tensor(
                out=m_scr[:], in0=iota_sb[:, lo:lo + W],
                scalar=tgt_sb[:, k:k + 1], in1=x_t[:, j, lo:lo + W],
                op0=ALU.is_equal, op1=ALU.mult,
                accum_out=g_cols[:, k:k + 1],
            )

        def act_accum(x_t, j, k):
            e_scr = scr.tile([P, C], F32, tag="e_scr")
            nc.scalar.activation(
                out=e_scr[:], in_=x_t[:, j, :], func=AF.Exp,
                accum_out=s_cols[:, k:k + 1],
            )

        def split_chunk(ch, sub, mixed=False):
            """Process chunk ch as quarter-DMAs of `sub` tiles.

            mixed=True alternates ACT-accum and exp+DVE-reduce per tile so
            back-to-back ACTIVATE+accum pairs don't serialize on the
            accumulator WAR (observed ~1.1us stall per pair in the drain).
            """
            for a in range(0, G, sub):
                x_t = xh.tile([P, sub, C], F32, tag="xh")
                nc.sync.dma_start(out=x_t[:], in_=x_h[ch][:, a:a + sub, :])
                for j in range(sub):
                    k = ch * G + a + j
                    if mixed and (j % 2 == 0):
                        e_one = scr.tile([P, 1, C], F32, tag="e_one")
                        nc.scalar.activation(
                            out=e_one[:], in_=x_t[:, j:j + 1, :], func=AF.Exp,
                        )
                        nc.vector.tensor_reduce(
                            out=s_cols[:, k:k + 1], in_=e_one[:],
                            axis=mybir.AxisListType.X, op=ALU.add,
                        )
                    else:
                        act_accum(x_t, j, k)
                    gather(x_t, j, k)

        # First chunk as quarter-DMAs so compute ramps immediately.
        split_chunk(0, 2)

        nc.sync.dma_start(out=tgt_sb[:], in_=tgt_h[:])
        nc.sync.dma_start(out=iota_sb[:], in_=iota_h[:])

        # Worst-class count depends only on tgt: do it up front.
        eq = pers.tile([P, N_TILES], F32, tag="eq")
        nc.vector.tensor_scalar(
            out=eq[:], in0=tgt_sb[:], scalar1=WORST[0], scalar2=None,
            op0=ALU.is_equal,
        )
        for v in WORST[1:-1]:
            nc.vector.scalar_tensor_tensor(
                out=eq[:], in0=tgt_sb[:], scalar=v, in1=eq[:],
                op0=ALU.is_equal, op1=ALU.add,
            )
        nc.vector.scalar_tensor_tensor(
            out=eq[:], in0=tgt_sb[:], scalar=WORST[-1], in1=eq[:],
            op0=ALU.is_equal, op1=ALU.add,
            accum_out=out_sb[:, 1:2],
        )

        nm = G - NA
        for ch in range(1, N_CHUNKS - 1):
            x_t = xp.tile([P, G, C], F32, tag="x_t")
            nc.sync.dma_start(out=x_t[:], in_=x_h[ch])
            k0 = ch * G
            # multi-row exp for tiles [0, nm) + one DVE reduce
            e_mul = em.tile([P, nm, C], F32, tag="e_mul")
            nc.scalar.activation(
                out=e_mul[:], in_=x_t[:, 0:nm, :], func=AF.Exp,
            )
            nc.vector.tensor_reduce(
                out=s_cols[:, k0:k0 + nm], in_=e_mul[:],
                axis=mybir.AxisListType.X, op=ALU.add,
            )
            for j in range(nm, G):
                act_accum(x_t, j, k0 + j)
            for j in range(G):
                gather(x_t, j, k0 + j)

        # Last chunk as quarter-DMAs, all-accum: shortest drain.
        split_chunk(N_CHUNKS - 1, 2, mixed=True)

        # fin0 = sum_k ln(s_k); fin1 = sum_k x_t,k
        lse_cols = pers.tile([P, N_TILES], F32, tag="lse_cols")
        nc.scalar.activation(
            out=lse_cols[:], in_=s_cols[:], func=AF.Ln,
            accum_out=fin[:, 0:1],
        )
        nc.vector.tensor_reduce(
            out=fin[:, 1:2], in_=g_cols[:], axis=mybir.AxisListType.X, op=ALU.add,
        )
        nc.vector.tensor_tensor(
            out=out_sb[:, 0:1], in0=fin[:, 0:1], in1=fin[:, 1:2], op=ALU.subtract,
        )

        nc.sync.dma_start(out=out_h[:], in_=out_sb[:])

    nc.compile()
    return nc


def _shard_inputs(output: np.ndarray, target: np.ndarray):
    in_maps = []
    los = np.array(LOS, dtype=np.int64)           # [N_TILES]
    iota = np.tile(np.arange(C, dtype=np.float32), (P, 1))
    for c in range(N_CORES):
        xs = output[c * ROWS:(c + 1) * ROWS]
        ts = target[c * ROWS:(c + 1) * ROWS]
        order = np.argsort(ts, kind="stable")
        xs = xs[order]
        ts = ts[order].astype(np.float32)
        # check every tile's targets fall inside its static window
        t_tiles = ts.reshape(N_TILES, P)          # tile k = sorted rows 128k..
        assert (t_tiles.min(axis=1) >= los).all() and \
               (t_tiles.max(axis=1) < los + W).all(), \
            "gather window violated; widen W"
        # layout: tile k=ch*G+j, partition p holds sorted row 128*k + p,
        # i.e. x[ch, p, j] = xs[ch*(G*P) + j*P + p]
        tgt = np.ascontiguousarray(t_tiles.T)     # [P, N_TILES]
        in_maps.append({
            "x": np.ascontiguousarray(
                xs.reshape(N_CHUNKS, G, P, C).transpose(0, 2, 1, 3)),
            "tgt": tgt,
            "iota": iota,
        })
    return in_maps


def _combine(results) -> np.float32:
    nll = 0.0
    cnt = 0.0
    for r in results:
        nll += float(r["out"][:, 0].astype(np.float64).sum())
        cnt += float(r["out"][:, 1].astype(np.float64).sum())
    loss = nll / B
    mask_mean = cnt / B
    return np.float32(loss * (1.0 + mask_mean))


def _run(in_maps, **kwargs):
    if "nc" not in _CACHE:
        _CACHE["nc"] = _build()
    return run_bass_kernel_spmd(_CACHE["nc"], in_maps, list(range(N_CORES)),
                                **kwargs)


def kernel(output: np.ndarray, target: np.ndarray) -> np.float32:
    assert output.shape == (B, C) and target.shape == (B,)
    res = _run(_shard_inputs(output, target))
    return _combine(res.results)


# revision 16
# speedup vs baseline: 1.2366x; 1.2366x over previous
"""Trainium2 Bass kernel for nn_CustomLoss (cross-entropy + worst-class masked loss).

Computes: loss = mean_i(logsumexp(output_i) - output_i[target_i])
          result = loss * (1 + mean_i(target_i in {3,5,8,9}))

Data-parallel over 8 NeuronCores: each core streams its 32768x1000 f32 shard
as 4 MB chunks. Rows are pre-sorted by target on the host (the loss is
permutation-invariant), so each [128,1000] tile's targets fall inside a
narrow static column window; the target-logit gather is a cheap windowed DVE
scalar_tensor_tensor over 32 columns. Row-sums of exp are split within each
chunk between the ACT accumulator (NA tiles) and one multi-row ACT exp +
DVE 3D tensor_reduce (G-NA tiles), keeping both compute engines ~15% under
the DMA streaming pace so the DMA rings stay the pacer. The first and last
chunks are split into quarter-DMAs to shorten pipeline ramp and drain.
Host combines the per-core partial sums.
"""
import os
import numpy as np
from contextlib import ExitStack

import concourse.bacc as bacc
import concourse.tile as tile
from concourse import mybir
from concourse.bass_utils import run_bass_kernel_spmd

F32 = mybir.dt.float32
AF = mybir.ActivationFunctionType
ALU = mybir.AluOpType

N_CORES = 8
B, C = 262144, 1000
ROWS = B // N_CORES           # 32768 rows per core
P = 128                       # SBUF partitions
G = 8                         # [128, C] sub-tiles per DMA chunk
N_CHUNKS = ROWS // (P * G)    # 32 chunks of [128, G, C] (4 MB contiguous)
N_TILES = ROWS // P           # 256 logical [128, C] tiles
W = 32                        # gather window width (covers sorted-target spread)
NA = int(os.environ.get("K_NA", "3"))   # ACT-accum tiles per chunk (of G)
BUFS_X = int(os.environ.get("K_BUFS", "3"))
WORST = (3.0, 5.0, 8.0, 9.0)

_CACHE = {}


def _window_starts():
    """Static per-tile gather window start columns.

    After sorting, tile k (sorted rows 128k..128k+127) has targets centered
    near 1000*(128k+64)/32768; the spread across 8 cores is a few classes.
    A +/-16 window covers it with margin; _shard_inputs asserts this
    against the actual data.
    """
    los = []
    for k in range(N_TILES):
        center = (128 * k + 64) * C // ROWS
        lo = min(max(center - W // 2, 0), C - W)
        los.append(lo)
    return los

LOS = _window_starts()


def _build():
    nc = bacc.Bacc(None, target_bir_lowering=False, debug=False,
                   num_devices=N_CORES)
    x_h = nc.declare_dram_parameter("x", [N_CHUNKS, P, G, C], F32, isOutput=False)
    tgt_h = nc.declare_dram_parameter("tgt", [P, N_TILES], F32, isOutput=False)
    iota_h = nc.declare_dram_parameter("iota", [P, C], F32, isOutput=False)
    out_h = nc.declare_dram_parameter("out", [P, 2], F32, isOutput=True)

    with tile.TileContext(nc) as tc, ExitStack() as ctx:
        xp = ctx.enter_context(tc.tile_pool(name="xp", bufs=BUFS_X))
        xh = ctx.enter_context(tc.tile_pool(name="xh", bufs=4))
        scr = ctx.enter_context(tc.tile_pool(name="scr", bufs=2))
        em = ctx.enter_context(tc.tile_pool(name="em", bufs=2))
        pers = ctx.enter_context(tc.tile_pool(name="pers", bufs=1))

        s_cols = pers.tile([P, N_TILES], F32, tag="s_cols")   # sum_j exp(x_ij)
        g_cols = pers.tile([P, N_TILES], F32, tag="g_cols")   # x_i[t_i]
        tgt_sb = pers.tile([P, N_TILES], F32, tag="tgt_sb")
        iota_sb = pers.tile([P, C], F32, tag="iota_sb")
        fin = pers.tile([P, 8], F32, tag="fin")
        out_sb = pers.tile([P, 2], F32, tag="out_sb")

        def gather(x_t, j, k):
            lo = LOS[k]
            m_scr = scr.tile([P, W], F32, tag="m_scr")
            nc.vector.scalar_tensor_tensor(
                out=m_scr[:], in0=iota_sb[:, lo:lo + W],
                scalar=tgt_sb[:, k:k + 1], in1=x_t[:, j, lo:lo + W],
                op0=ALU.is_equal, op1=ALU.mult,
                accum_out=g_cols[:, k:k + 1],
            )

        def act_accum(x_t, j, k):
            e_scr = scr.tile([P, C], F32, tag="e_scr")
            nc.scalar.activation(
                out=e_scr[:], in_=x_t[:, j, :], func=AF.Exp,
                accum_out=s_cols[:, k:k + 1],
            )

        def split_chunk(ch, sub, mixed=False):
            """Process chunk ch as quarter-DMAs of `sub` tiles.

            mixed=True alternates ACT-accum and exp+DVE-reduce per tile so
            back-to-back ACTIVATE+accum pairs don't serialize on the
            accumulator WAR (observed ~1.1us stall per pair in the drain).
            """
            for a in range(0, G, sub):
                x_t = xh.tile([P, sub, C], F32, tag="xh")
                nc.sync.dma_start(out=x_t[:], in_=x_h[ch][:, a:a + sub, :])
                for j in range(sub):
                    k = ch * G + a + j
                    if mixed and (j % 2 == 0):
                        e_one = scr.tile([P, 1, C], F32, tag="e_one")
                        nc.scalar.activation(
                            out=e_one[:], in_=x_t[:, j:j + 1, :], func=AF.Exp,
                        )
                        nc.vector.tensor_reduce(
                            out=s_cols[:, k:k + 1], in_=e_one[:],
                            axis=mybir.AxisListType.X, op=ALU.add,
                        )
                    else:
                        act_accum(x_t, j, k)
                    gather(x_t, j, k)

        # First chunk as quarter-DMAs so compute ramps immediately.
        split_chunk(0, 2)

        nc.sync.dma_start(out=tgt_sb[:], in_=tgt_h[:])
        nc.sync.dma_start(out=iota_sb[:], in_=iota_h[:])

        # Worst-class count depends only on tgt: do it up front.
        eq = pers.tile([P, N_TILES], F32, tag="eq")
        nc.vector.tensor_scalar(
            out=eq[:], in0=tgt_sb[:], scalar1=WORST[0], scalar2=None,
            op0=ALU.is_equal,
        )
        for v in WORST[1:-1]:
            nc.vector.scalar_tensor_tensor(
                out=eq[:], in0=tgt_sb[:], scalar=v, in1=eq[:],
                op0=ALU.is_equal, op1=ALU.add,
            )
        nc.vector.scalar_tensor_tensor(
            out=eq[:], in0=tgt_sb[:], scalar=WORST[-1], in1=eq[:],
            op0=ALU.is_equal, op1=ALU.add,
            accum_out=out_sb[:, 1:2],
        )

        nm = G - NA
        for ch in range(1, N_CHUNKS - 1):
            x_t = xp.tile([P, G, C], F32, tag="x_t")
            nc.sync.dma_start(out=x_t[:], in_=x_h[ch])
            k0 = ch * G
            # multi-row exp for tiles [0, nm) + one DVE reduce
            e_mul = em.tile([P, nm, C], F32, tag="e_mul")
            nc.scalar.activation(
                out=e_mul[:], in_=x_t[:, 0:nm, :], func=AF.Exp,
            )
            nc.vector.tensor_reduce(
                out=s_cols[:, k0:k0 + nm], in_=e_mul[:],
                axis=mybir.AxisListType.X, op=ALU.add,
            )
            for j in range(nm, G):
                act_accum(x_t, j, k0 + j)
            for j in range(G):
                gather(x_t, j, k0 + j)

        # Last chunk as quarter-DMAs, all-accum: shortest drain.
        split_chunk(N_CHUNKS - 1, 2)

        # fin0 = sum_k ln(s_k); fin1 = sum_k x_t,k
        lse_cols = pers.tile([P, N_TILES], F32, tag="lse_cols")
        nc.scalar.activation(
            out=lse_cols[:], in_=s_cols[:], func=AF.Ln,
            accum_out=fin[:, 0:1],
        )
        nc.vector.tensor_reduce(
            out=fin[:, 1:2], in_=g_cols[:], axis=mybir.AxisListType.X, op=ALU.add,
        )
        nc.vector.tensor_tensor(
            out=out_sb[:, 0:1], in0=fin[:, 0:1], in1=fin[:, 1:2], op=ALU.subtract,
        )

        nc.sync.dma_start(out=out_h[:], in_=out_sb[:])

    nc.compile()
    return nc


def _shard_inputs(output: np.ndarray, target: np.ndarray):
    in_maps = []
    los = np.array(LOS, dtype=np.int64)           # [N_TILES]
    iota = np.tile(np.arange(C, dtype=np.float32), (P, 1))
    for c in range(N_CORES):
        xs = output[c * ROWS:(c + 1) * ROWS]
        ts = target[c * ROWS:(c + 1) * ROWS]
        order = np.argsort(ts, kind="stable")
        xs = xs[order]
        ts = ts[order].astype(np.float32)
        # check every tile's targets fall inside its static window
        t_tiles = ts.reshape(N_TILES, P)          # tile k = sorted rows 128k..
        assert (t_tiles.min(axis=1) >= los).all() and \
               (t_tiles.max(axis=1) < los + W).all(), \
            "gather window violated; widen W"
        # layout: tile k=ch*G+j, partition p holds sorted row 128*k + p,
        # i.e. x[ch, p, j] = xs[ch*(G*P) + j*P + p]
        tgt = np.ascontiguousarray(t_tiles.T)     # [P, N_TILES]
        in_maps.append({
            "x": np.ascontiguousarray(
                xs.reshape(N_CHUNKS, G, P, C).transpose(0, 2, 1, 3)),
            "tgt": tgt,
            "iota": iota,
        })
    return in_maps


def _combine(results) -> np.float32:
    nll = 0.0
    cnt = 0.0
    for r in results:
        nll += float(r["out"][:, 0].astype(np.float64).sum())
        cnt += float(r["out"][:, 1].astype(np.float64).sum())
    loss = nll / B
    mask_mean = cnt / B
    return np.float32(loss * (1.0 + mask_mean))


def _run(in_maps, **kwargs):
    if "nc" not in _CACHE:
        _CACHE["nc"] = _build()
    return run_bass_kernel_spmd(_CACHE["nc"], in_maps, list(range(N_CORES)),
                                **kwargs)


def kernel(output: np.ndarray, target: np.ndarray) -> np.float32:
    assert output.shape == (B, C) and target.shape == (B,)
    res = _run(_shard_inputs(output, target))
    return _combine(res.results)
